# revision 2
# baseline (speedup 1.0000x reference)
"""Cross-modal MHA on 8 NeuronCores — fp8 DoubleRow edition.

- data-parallel over batch: 1 element per core, no collectives
- host splits q/k/v and Wq/Wk/Wv into fp8e4m3 hi+lo pairs (hi = fp8(x),
  lo = fp8(x - hi)); projections run as 3-product DoubleRow matmuls
  (hi*hi + lo*hi + hi*lo, lolo dropped) at 0.5 cycles/row
- scores: qh/kh re-split to fp8 hi/lo on device, partition-stacked
  [hi(0:64); lo(64:128)]; ONE DoubleRow matmul per kv-tile computes the
  full 4-product at 0.5 cycles/row (2x bf16)
- PV flipped: out[q,dk] with lhsT=e[kv,q-chunk], rhs=vh[kv,65] (ones col
  -> denominators per q partition); per-partition reciprocal+scale on DVE
- attn out transposed back to [dm,q] via PE-transpose (identity rhs) for
  the bf16 out-projection
"""

import numpy as np

HEADS = 16
DM = 1024
IMG = 512
DK = 64
LQ = 2048
LKV = 1024
B = 8
P = 128
IB = 512
HW = DK + 1

_cache = {}
TRACE = False
LAST_RESULT = None


def _build_nc(lq=LQ, lkv=LKV):
    from contextlib import ExitStack

    import concourse.tile as tile
    from concourse import bacc, mybir

    dt = mybir.dt
    f32 = dt.float32
    bf16 = dt.bfloat16
    fp8 = dt.float8e4
    DR = mybir.MatmulPerfMode.DoubleRow
    Exp = mybir.ActivationFunctionType.Exp
    Alu = mybir.AluOpType

    n_ib = lq // IB          # 4 q blocks
    n_jt = lkv // P          # 8 kv tiles
    n_mc = DM // P           # 8 dm-out chunks
    n_kq = DM // P           # 8 contraction chunks (q proj)
    n_kk = IMG // P          # 4 contraction chunks (k/v proj)

    nc = bacc.Bacc("TRN2", target_bir_lowering=False, debug=False)

    qT = nc.declare_dram_parameter("qT", [P, n_kq, 2, lq], fp8, isOutput=False)
    kT = nc.declare_dram_parameter("kT", [P, n_kk, 2, lkv], fp8, isOutput=False)
    vT = nc.declare_dram_parameter("vT", [P, n_kk, 2, lkv], fp8, isOutput=False)
    Wq = nc.declare_dram_parameter("Wq", [P, n_mc, n_kq, 2, P], fp8, isOutput=False)
    Wk = nc.declare_dram_parameter("Wk", [P, n_mc, n_kk, 2, P], fp8, isOutput=False)
    Wv = nc.declare_dram_parameter("Wv", [P, n_kk, 2, DM], fp8, isOutput=False)
    Wo = nc.declare_dram_parameter("Wo", [P, n_mc, DM], bf16, isOutput=False)
    bq = nc.declare_dram_parameter("bq", [DM], f32, isOutput=False)
    bk = nc.declare_dram_parameter("bk", [DM], f32, isOutput=False)
    bv = nc.declare_dram_parameter("bv", [DM], f32, isOutput=False)
    bo = nc.declare_dram_parameter("bo", [DM], f32, isOutput=False)
    idn = nc.declare_dram_parameter("idn", [P, P], bf16, isOutput=False)
    out = nc.declare_dram_parameter("out", [lq, DM], f32, isOutput=True)

    with tile.TileContext(nc) as tc, ExitStack() as ctx:
        singles = ctx.enter_context(tc.tile_pool(name="singles", bufs=1))

        psS = ctx.enter_context(tc.tile_pool(name="psS", bufs=2, space="PSUM"))
        psP = ctx.enter_context(tc.tile_pool(name="psP", bufs=2, space="PSUM"))
        psV = ctx.enter_context(tc.tile_pool(name="psV", bufs=1, space="PSUM"))
        psT = ctx.enter_context(tc.tile_pool(name="psT", bufs=1, space="PSUM"))

        kT_sb = singles.tile([P, n_kk, 2, lkv], fp8)
        vT_sb = singles.tile([P, n_kk, 2, lkv], fp8)
        Wq_sb = singles.tile([P, n_mc, n_kq, 2, P], fp8)
        Wk_sb = singles.tile([P, n_mc, n_kk, 2, P], fp8)
        Wv_sb = singles.tile([P, n_kk, 2, DM], fp8)
        Wo_sb = singles.tile([P, n_mc, DM], bf16)
        bq_sb = singles.tile([P, n_mc], f32)
        bqo_sb = singles.tile([64, n_mc], f32)
        bk_sb = singles.tile([P, n_mc], f32)
        bv_rep = singles.tile([P, DM], f32)
        bo_rep = singles.tile([P, DM], f32)
        idn_sb = singles.tile([P, P], bf16)
        kh_st = singles.tile([P, HEADS, n_jt, 2, P], fp8)
        vh_sb = singles.tile([P, n_jt, HEADS * HW], bf16)

        e_pool = ctx.enter_context(tc.tile_pool(name="e", bufs=8))
        qTb_pool = ctx.enter_context(tc.tile_pool(name="qTb", bufs=2))
        qst_pool = ctx.enter_context(tc.tile_pool(name="qst", bufs=2))
        outT_pool = ctx.enter_context(tc.tile_pool(name="outT", bufs=2))
        pair_pool = ctx.enter_context(tc.tile_pool(name="pair", bufs=2))
        osb_pool = ctx.enter_context(tc.tile_pool(name="osb", bufs=4))
        rec_pool = ctx.enter_context(tc.tile_pool(name="rec", bufs=4))
        kst_pool = ctx.enter_context(tc.tile_pool(name="kst", bufs=2))

        # ---------- projection emitters ----------
        def kp_chunk(mc, kstage):
            """K-proj for dm-out chunk mc (= head pair), both kv halves,
            into staging [128,{hi,lo},1024] fp8."""
            def run():
                for jb in range(2):
                    pp = psP.tile([P, IB], f32, tag="psP", name=f"kp{mc}_{jb}")
                    sl = slice(jb * IB, (jb + 1) * IB)
                    for c in range(n_kk):
                        nc.tensor.matmul(
                            pp, lhsT=Wk_sb[:, mc, c, 0:2, :],
                            rhs=kT_sb[:, c, 0:1, sl].to_broadcast([P, 2, IB]),
                            perf_mode=DR, start=(c == 0), stop=False)
                    for j in range(n_kk // 2):
                        nc.tensor.matmul(
                            pp, lhsT=Wk_sb[:, mc, 2 * j:2 * j + 2, 0, :],
                            rhs=kT_sb[:, 2 * j:2 * j + 2, 1, sl],
                            perf_mode=DR, start=False, stop=(j == n_kk // 2 - 1))
                    bsc = bk_sb[:, mc:mc + 1]
                    nc.vector.tensor_scalar(
                        kstage[:, 0, sl], pp, 0.0625, bsc, Alu.mult, Alu.add)
                    nc.vector.scalar_tensor_tensor(
                        kstage[:, 1, sl], in0=pp, scalar=0.0625,
                        in1=kstage[:, 0, sl], op0=Alu.mult, op1=Alu.subtract)
            return run

        def kst_copies(mc, kstage):
            """Build stacked kh tiles for heads 2mc/2mc+1 from staging."""
            def run():
                for par in range(2):
                    h = 2 * mc + par
                    sp = slice(64 * par, 64 * par + 64)
                    hi = kstage[sp, 0, :].rearrange("p (t j) -> p t j", j=P)
                    lo = kstage[sp, 1, :].rearrange("p (t j) -> p t j", j=P)
                    nc.sync.dma_start(kh_st[0:64, h, :, 0, :], hi)
                    nc.sync.dma_start(kh_st[64:128, h, :, 0, :], lo)
                    nc.sync.dma_start(kh_st[0:64, h, :, 1, :], lo)
                    nc.sync.dma_start(kh_st[64:128, h, :, 1, :], hi)
            return run

        def vp_chunk(jt, db):
            def run():
                pp = psP.tile([P, IB], f32, tag="psP", name=f"vp{jt}_{db}")
                jsl = slice(jt * P, (jt + 1) * P)
                dsl = slice(db * IB, (db + 1) * IB)
                for c in range(n_kk):
                    nc.tensor.matmul(
                        pp, lhsT=vT_sb[:, c, 0:2, jsl],
                        rhs=Wv_sb[:, c, 0:1, dsl].to_broadcast([P, 2, IB]),
                        perf_mode=DR, start=(c == 0), stop=False)
                for j in range(n_kk // 2):
                    nc.tensor.matmul(
                        pp, lhsT=vT_sb[:, 2 * j:2 * j + 2, 0, jsl],
                        rhs=Wv_sb[:, 2 * j:2 * j + 2, 1, dsl],
                        perf_mode=DR, start=False, stop=(j == n_kk // 2 - 1))
                dst = vh_sb[:, jt, :].rearrange("p (h c) -> p h c", c=HW)[
                    :, db * 8:(db + 1) * 8, :DK]
                nc.vector.scalar_tensor_tensor(
                    dst, in0=pp.rearrange("p (h d) -> p h d", d=DK),
                    scalar=0.0625,
                    in1=bv_rep[:, dsl].rearrange("p (h d) -> p h d", d=DK),
                    op0=Alu.mult, op1=Alu.add)
            return run

        def make_qprep(ib):
            """Q-proj for block ib -> stacked qh tiles [128,{h},512] fp8."""
            i0 = ib * IB
            qTb = qTb_pool.tile([P, n_kq, 2, IB], fp8, tag="qTb",
                                name=f"qTb{ib}")
            nc.sync.dma_start(qTb, qT[:, :, :, i0:i0 + IB])
            qst = qst_pool.tile([P, HEADS, IB], fp8, tag="qst", name=f"qst{ib}")
            chunks = []

            def qp_chunk(mc):
                def run():
                    pp = psP.tile([P, IB], f32, tag="psP", name=f"qp{ib}_{mc}")
                    for c in range(n_kq):
                        nc.tensor.matmul(
                            pp, lhsT=Wq_sb[:, mc, c, 0:2, :],
                            rhs=qTb[:, c, 0:1, :].to_broadcast([P, 2, IB]),
                            perf_mode=DR, start=(c == 0), stop=False)
                    for j in range(n_kq // 2):
                        nc.tensor.matmul(
                            pp, lhsT=Wq_sb[:, mc, 2 * j:2 * j + 2, 0, :],
                            rhs=qTb[:, 2 * j:2 * j + 2, 1, :],
                            perf_mode=DR, start=False, stop=(j == n_kq // 2 - 1))
                    for par in range(2):
                        h = 2 * mc + par
                        sp = slice(64 * par, 64 * par + 64)
                        bsc = bq_sb[sp, mc:mc + 1]
                        nc.vector.tensor_scalar(
                            qst[0:64, h, :], pp[sp, :], 0.0625, bsc,
                            Alu.mult, Alu.add)
                        nc.vector.scalar_tensor_tensor(
                            qst[64:128, h, :], in0=pp[sp, :], scalar=0.0625,
                            in1=qst[0:64, h, :], op0=Alu.mult,
                            op1=Alu.subtract)
                return run

            for mc in range(n_mc):
                chunks.append(qp_chunk(mc))
            return qst, chunks

        # ---------- attention emitters ----------
        def sc_head(qst, h, ib):
            """Scores+exp for head h: 2 psum groups of 4 kv-tiles."""
            es = []
            rhs = qst[:, h:h + 1, :].to_broadcast([P, 2, IB])

            def sc(g):
                ps = psS.tile([P, 2, IB], f32, tag="psS", name=f"s{ib}_{h}_{g}")
                for u in range(2):
                    t = 2 * g + u
                    nc.tensor.matmul(ps[:, u, :], lhsT=kh_st[:, h, t, :, :],
                                     rhs=rhs, perf_mode=DR,
                                     start=True, stop=True)
                e = e_pool.tile([P, 2, IB], bf16, tag="e", name=f"e{ib}_{h}_{g}")
                nc.scalar.activation(e, ps, Exp, scale=0.125)
                es.append(e)
            return sc, es

        def tr_pair(pair, outT, mc, ib):
            """Transpose pair tile [128q,128dm] -> outT[:, mc, :]."""
            def tr(qc):
                pt = psT.tile([P, 2, P], bf16, tag="psT", name=f"t{ib}_{mc}")
                nc.tensor.matmul(pt[:, qc % 2, :], lhsT=pair[:, qc, :],
                                 rhs=idn_sb, is_transpose=True,
                                 start=True, stop=True)
                nc.vector.tensor_copy(outT[:, mc, qc * P:(qc + 1) * P],
                                      pt[:, qc % 2, :])
            return tr

        def make_final(outT, ib):
            i0 = ib * IB
            chunks = []

            def fin(qc, db):
                def run():
                    pf = psP.tile([P, IB], f32, tag="psP", name=f"f{ib}_{qc}_{db}")
                    dsl = slice(db * IB, (db + 1) * IB)
                    for mc in range(n_mc):
                        nc.tensor.matmul(
                            pf, lhsT=outT[:, mc, qc * P:(qc + 1) * P],
                            rhs=Wo_sb[:, mc, dsl],
                            start=(mc == 0), stop=(mc == n_mc - 1))
                    osb = osb_pool.tile([P, IB], f32, tag="osb",
                                        name=f"o{ib}_{qc}_{db}")
                    nc.vector.tensor_tensor(osb, pf, bo_rep[:, dsl], Alu.add)
                    nc.sync.dma_start(
                        out[i0 + qc * P:i0 + (qc + 1) * P, dsl], osb)
                return run

            for qc in range(4):
                for db in range(2):
                    chunks.append(fin(qc, db))
            return chunks

        # ---------- emission ----------
        nc.sync.dma_start(bk_sb, bk.rearrange("(o p) -> p o", p=P))
        nc.sync.dma_start(bq_sb, bq.rearrange("(o p) -> p o", p=P))
        nc.sync.dma_start(bqo_sb, bq.rearrange("(o p) -> p o", p=P)[64:128, :])
        nc.sync.dma_start(bv_rep, bv[None, :].to_broadcast([P, DM]))
        nc.sync.dma_start(Wv_sb, Wv[:, :, :, :])
        nc.sync.dma_start(vT_sb, vT[:, :, :, :])
        nc.sync.dma_start(Wk_sb, Wk[:, :, :, :, :])
        nc.sync.dma_start(kT_sb, kT[:, :, :, :])
        nc.sync.dma_start(Wq_sb, Wq[:, :, :, :, :])
        nc.sync.dma_start(bo_rep, bo[None, :].to_broadcast([P, DM]))
        nc.sync.dma_start(idn_sb, idn[:, :])

        # startup: interleave K-proj and Q-proj block 0 so PE never waits
        # on a single chunk's DVE drain (psP WAR distance = 2)
        qst0, qprep0 = make_qprep(0)
        for jt in range(n_jt):
            for db in range(2):
                vp_chunk(jt, db)()
        ones_view = vh_sb.rearrange("p o (h c) -> p o h c", c=HW)[:, :, :, DK]
        nc.vector.memset(ones_view, 1.0)

        def kq_startup(mc):
            ks = kst_pool.tile([P, 2, lkv], fp8, tag="kst", name=f"ks{mc}")
            kp_chunk(mc, ks)()
            qprep0[mc]()
            kst_copies(mc, ks)()
        kq_startup(0)

        # block loop: software pipeline
        qst_cur = qst0
        cur_jit = []
        pending = []
        for ib in range(n_ib):
            outT = outT_pool.tile([P, n_mc, IB], bf16, tag="outT",
                                  name=f"oT{ib}")
            if ib + 1 < n_ib:
                qst_next, qprep_next = make_qprep(ib + 1)
            else:
                qst_next, qprep_next = None, []
            head_extras = qprep_next[0:2]
            next_jit = qprep_next[2:]
            extras = []
            for i in range(max(len(head_extras), len(pending))):
                if i < len(head_extras):
                    extras.append(head_extras[i])
                if i < len(pending):
                    extras.append(pending[i])
            per = (len(extras) + HEADS - 1) // HEADS
            ei = 0
            prev = None  # PV+norm+transpose emitters for previous head
            for h in range(HEADS):
                if ib == 0 and h % 2 == 0 and h // 2 + 1 < n_mc:
                    kq_startup(h // 2 + 1)
                if ib == 0 and h == 4:
                    nc.sync.dma_start(Wo_sb, Wo[:, :, :])
                if h % 2 == 0 and h // 2 < len(cur_jit):
                    cur_jit[h // 2]()
                sc, es = sc_head(qst_cur, h, ib)
                sc(0)
                sc(1)
                if prev is not None:
                    prev(0, 2)
                sc(2)
                sc(3)
                if prev is not None:
                    prev(2, 4)
                if h % 2 == 0:
                    pair = pair_pool.tile([P, 4, P], bf16, tag="pair",
                                          name=f"pr{ib}_{h // 2}")

                def mk_prev(es=es, h=h, pair=pair, ib=ib):
                    par = h % 2
                    pvs = psV.tile([P, 4, HW], f32, tag="psV",
                                   name=f"pv{ib}_{h}")
                    rec = rec_pool.tile([P, 4], f32, tag="rec",
                                        name=f"rc{ib}_{h}")

                    def run(q0, q1):
                        for qc in range(q0, q1):
                            qsl = slice(qc * P, (qc + 1) * P)
                            for jt in range(n_jt):
                                nc.tensor.matmul(
                                    pvs[:, qc, :],
                                    lhsT=es[jt // 2][:, jt % 2, qsl],
                                    rhs=vh_sb[:, jt, h * HW:(h + 1) * HW],
                                    start=(jt == 0), stop=(jt == n_jt - 1))
                            nc.vector.reciprocal(rec[:, qc:qc + 1],
                                                 pvs[:, qc, DK:DK + 1])
                            nc.vector.tensor_scalar(
                                pair[:, qc, par * DK:par * DK + DK],
                                pvs[:, qc, 0:DK], rec[:, qc:qc + 1],
                                None, Alu.mult)
                            if par == 1:
                                tr_pair(pair, outT, h // 2, ib)(qc)
                    return run

                prev = mk_prev()
                for _ in range(per):
                    if ei < len(extras):
                        extras[ei]()
                        ei += 1
            prev(0, 4)
            while ei < len(extras):
                extras[ei]()
                ei += 1
            pending = make_final(outT, ib)
            qst_cur = qst_next
            cur_jit = next_jit
        for ch in pending:
            ch()

    nc.compile()
    return nc


def _get_nc(lq=LQ, lkv=LKV):
    key = (lq, lkv)
    if key not in _cache:
        _cache[key] = _build_nc(lq, lkv)
    return _cache[key]


def _host_pack(inputs):
    import ml_dtypes

    bf16 = ml_dtypes.bfloat16
    f8 = ml_dtypes.float8_e4m3fn

    def split8(x):
        hi = x.astype(f8)
        lo = (x - hi.astype(np.float32)).astype(f8)
        return hi, lo

    shared = {}
    Wq = np.asarray(inputs["Wq"], dtype=np.float32)
    Wk = np.asarray(inputs["Wk"], dtype=np.float32)
    Wv = np.asarray(inputs["Wv"], dtype=np.float32)
    Wo = np.asarray(inputs["Wo"], dtype=np.float32)

    def pack_w(W, n_kc):
        hi, lo = split8(W * 16.0)  # [KIN, 1024], x16: keep lo out of subnormals
        # -> [128, mc 8, kc, 2, 128]
        arr = np.empty((P, DM // P, n_kc, 2, P), dtype=f8)
        h4 = hi.reshape(n_kc, P, DM // P, P)  # [kc, p, mc, j]
        l4 = lo.reshape(n_kc, P, DM // P, P)
        arr[:, :, :, 0, :] = h4.transpose(1, 2, 0, 3)
        arr[:, :, :, 1, :] = l4.transpose(1, 2, 0, 3)
        return np.ascontiguousarray(arr)

    shared["Wq"] = pack_w(Wq, 8)
    shared["Wk"] = pack_w(Wk, 4)
    # Wv as rhs: [128, kc 4, 2, 1024]
    hi, lo = split8(Wv * 16.0)
    wv = np.empty((P, 4, 2, DM), dtype=f8)
    wv[:, :, 0, :] = hi.reshape(4, P, DM).transpose(1, 0, 2)
    wv[:, :, 1, :] = lo.reshape(4, P, DM).transpose(1, 0, 2)
    shared["Wv"] = np.ascontiguousarray(wv)
    shared["Wo"] = np.ascontiguousarray(
        Wo.astype(bf16).reshape(8, P, DM).transpose(1, 0, 2))
    for n in ("bq", "bk", "bv", "bo"):
        shared[n] = np.ascontiguousarray(np.asarray(inputs[n], np.float32))
    shared["idn"] = np.eye(P, dtype=np.float32).astype(bf16)

    def pack_xT(x, n_kc, L):
        hi, lo = split8(x.T.astype(np.float32))  # [D, L]
        arr = np.empty((P, n_kc, 2, L), dtype=f8)
        arr[:, :, 0, :] = hi.reshape(n_kc, P, L).transpose(1, 0, 2)
        arr[:, :, 1, :] = lo.reshape(n_kc, P, L).transpose(1, 0, 2)
        return np.ascontiguousarray(arr)

    in_maps = []
    q = np.asarray(inputs["q"], dtype=np.float32)
    k = np.asarray(inputs["k"], dtype=np.float32)
    v = np.asarray(inputs["v"], dtype=np.float32)
    for b in range(B):
        m = dict(shared)
        m["qT"] = pack_xT(q[b], 8, LQ)
        m["kT"] = pack_xT(k[b], 4, LKV)
        m["vT"] = pack_xT(v[b], 4, LKV)
        in_maps.append(m)
    return in_maps


def kernel(**inputs):
    from concourse.bass_utils import run_bass_kernel_spmd

    nc = _get_nc()
    in_maps = _host_pack(inputs)
    res = run_bass_kernel_spmd(nc, in_maps, list(range(B)), trace=TRACE)
    global LAST_RESULT
    LAST_RESULT = res
    return np.stack([res.results[b]["out"] for b in range(B)], axis=0)
